# revision 15
# baseline (speedup 1.0000x reference)
"""Trainium2 Bass kernel for nn_AttentionKernel_89455578841177.

Multi-head attention: qkv = node @ W_qkv; softmax(q k^T / sqrt(D)) v; out @ W_out.
B=2, S=2048, E=1024, H=16, D=64.

Sharding over 8 NeuronCores: data parallel on B (2) x tensor parallel on heads
(16 heads -> 4 groups of 4). Each core computes a per-head-group partial of the
output projection; the host sums the 4 partials per batch element.

Device-side structure per core (all matmuls bf16 inputs, fp32 PSUM accumulate):
  phase 1: qT = (x Wq)^T, kT = (x Wk)^T  in [d, s] layout;  v = x Wv in [s, d]
           layout with a ones-column appended per head (for softmax row sums).
  phase 2: per head pair (row-group-tiled K=64 matmuls run concurrently):
           s^T = k q^T  -> exp on ScalarE -> p^T;   [o^T | r] = [v|1]^T-style
           matmul accumulated over k-chunks in PSUM;  normalize by 1/r.
  phase 3: y_partial = a W_out  (a = concatenated normalized heads).
The 1/sqrt(D) scale is folded into Wq on the host (exact: power of two).
Softmax skips the max-subtraction: scores are ~N(0,1) so exp cannot overflow.
"""

import numpy as np
import ml_dtypes

import concourse.bass as bass
import concourse.mybir as mybir
import concourse.tile as tile
from concourse import bacc
from concourse.bass_utils import run_bass_kernel_spmd

B, S, E = 2, 2048, 1024
H, D = 16, 64
NCORES = 8
GH = 4            # heads per core
GD = GH * D       # 256 = per-core slice of the head dim
P = 128
EO = E // P       # 8 contraction chunks for the projections
SC = S // P       # 16 s-chunks
MC = GD // P      # 2 head-pair chunks (2 heads of 64 rows per chunk)
NQ = 512          # matmul moving free dim
QB = 512          # Sq block size in phase 2
KV = D + 1        # v columns + ones column

BF = mybir.dt.bfloat16
FP = mybir.dt.float32
EXP = mybir.ActivationFunctionType.Exp

import os
_VARIANT = os.environ.get("KERNEL_VARIANT", "")


def _build_kernel(nc: bass.Bass, tc: tile.TileContext):
    xT = nc.dram_tensor("xT", [E, S], BF, kind="ExternalInput")
    wq = nc.dram_tensor("wq", [E, GD], BF, kind="ExternalInput")
    wk = nc.dram_tensor("wk", [E, GD], BF, kind="ExternalInput")
    wv = nc.dram_tensor("wv", [E, GD], BF, kind="ExternalInput")
    wo = nc.dram_tensor("wo", [GD, E], BF, kind="ExternalInput")
    y = nc.dram_tensor("y", [S, E], FP, kind="ExternalOutput")

    with (
        tc.tile_pool(name="const", bufs=1) as const,
        tc.tile_pool(name="pwork", bufs=4) as pwork,
        tc.tile_pool(name="evac", bufs=3) as evac,
    ):
        # ---- SBUF residents -------------------------------------------------
        x_sb = const.tile([P, EO, S], BF, tag="x")
        xT_r = xT.rearrange("(eo p) s -> p eo s", p=P)
        for eo in range(EO):
            nc.sync.dma_start(out=x_sb[:, eo, :], in_=xT_r[:, eo, :])

        wq_sb = const.tile([P, EO, GD], BF, tag="wq")
        nc.sync.dma_start(out=wq_sb, in_=wq.rearrange("(eo p) m -> p eo m", p=P))
        wk_sb = const.tile([P, EO, GD], BF, tag="wk")
        nc.sync.dma_start(out=wk_sb, in_=wk.rearrange("(eo p) m -> p eo m", p=P))
        wv_sb = const.tile([P, EO, GD], BF, tag="wv")
        nc.sync.dma_start(out=wv_sb, in_=wv.rearrange("(eo p) m -> p eo m", p=P))
        wo_sb = const.tile([P, MC, E], BF, tag="wo")
        nc.sync.dma_start(out=wo_sb, in_=wo.rearrange("(mc p) e -> p mc e", p=P))

        qT_sb = const.tile([P, MC, S], BF, tag="qT")
        kT_sb = const.tile([P, MC, S], BF, tag="kT")
        at_sb = const.tile([P, MC, S], BF, tag="at")   # normalized attn out^T
        v_sb = const.tile([P, SC, GH, KV], BF, tag="v")
        nc.vector.memset(v_sb[:, :, :, D : D + 1], 1.0)
        ones_sb = const.tile([1, 64], BF, tag="ones")
        nc.vector.memset(ones_sb, 1.0)

        # ---- phase 1: projections ------------------------------------------
        with tc.tile_pool(name="ps1", bufs=4, space="PSUM") as ps1:
            for wsrc, dst in ((wq_sb, qT_sb), (wk_sb, kT_sb)):
                for mc in range(MC):
                    for sq in range(S // NQ):
                        pst = ps1.tile([P, NQ], FP, tag="ps1")
                        for eo in range(EO):
                            nc.tensor.matmul(
                                pst,
                                lhsT=wsrc[:, eo, mc * P : (mc + 1) * P],
                                rhs=x_sb[:, eo, sq * NQ : (sq + 1) * NQ],
                                start=(eo == 0),
                                stop=(eo == EO - 1),
                            )
                        nc.vector.tensor_copy(
                            out=dst[:, mc, sq * NQ : (sq + 1) * NQ], in_=pst
                        )
            for sc in range(SC):
                psv = ps1.tile([P, NQ], FP, tag="ps1")
                for eo in range(EO):
                    nc.tensor.matmul(
                        psv[:, :GD],
                        lhsT=x_sb[:, eo, sc * P : (sc + 1) * P],
                        rhs=wv_sb[:, eo, :],
                        start=(eo == 0),
                        stop=(eo == EO - 1),
                    )
                nc.vector.tensor_copy(
                    out=v_sb[:, sc, :, 0:D],
                    in_=psv[:, :GD].rearrange("p (h d) -> p h d", h=GH),
                )

        # ---- phase 2: attention per head pair ------------------------------
        with (
            tc.tile_pool(name="ps_sc", bufs=2, space="PSUM") as ps_sc,
            tc.tile_pool(name="ps_pv", bufs=4, space="PSUM") as ps_pv,
        ):
            for mc in range(MC):
                for hf in range(S // QB):
                    q0 = hf * QB
                    po0 = ps_pv.tile([KV, QB], FP, tag="po")
                    po1 = ps_pv.tile([KV, QB], FP, tag="po")
                    po = (po0, po1)
                    for kc in range(SC):
                        # head pair packed side by side: one fp32 PSUM bank per
                        # head, K=64 row-tiled matmuls run concurrently
                        st = ps_sc.tile([P, 2 * QB], FP, tag="st")
                        for h in range(2):
                            hb = h * 64
                            nc.tensor.matmul(
                                st[:, h * QB : (h + 1) * QB],
                                lhsT=kT_sb[hb : hb + 64, mc, kc * P : (kc + 1) * P],
                                rhs=qT_sb[hb : hb + 64, mc, q0 : q0 + QB],
                                start=True,
                                stop=True,
                            )
                        pt = pwork.tile([P, 2 * QB], BF, tag="p")
                        nc.scalar.activation(pt, st, EXP)
                        for h in range(2):
                            nc.tensor.matmul(
                                po[h],
                                lhsT=v_sb[:, kc, mc * 2 + h, :],
                                rhs=pt[:, h * QB : (h + 1) * QB],
                                start=(kc == 0),
                                stop=(kc == SC - 1),
                                skip_group_check=True,
                            )
                    for h in range(2):
                        hb = h * 64
                        if _VARIANT == "nonorm":
                            nc.vector.tensor_copy(
                                out=at_sb[hb : hb + 64, mc, q0 : q0 + QB],
                                in_=po[h][0:D, :],
                            )
                            continue
                        rinv = evac.tile([1, QB], BF, tag="rinv")
                        with nc.allow_low_precision(
                            reason="1/rowsum broadcast scalar; bf16 is plenty"
                        ):
                            nc.vector.reciprocal(rinv, po[h][D : D + 1, :])
                        # broadcast 1/r across 64 partitions with a K=1 matmul
                        # (ones [1,64] outer-product; SBUF APs cannot have a
                        # zero partition step so DVE cannot broadcast rows)
                        rb_ps = ps_pv.tile([64, QB], FP, tag="po")
                        nc.tensor.matmul(
                            rb_ps, lhsT=ones_sb, rhs=rinv, start=True, stop=True
                        )
                        rb = evac.tile([64, QB], BF, tag="rb")
                        nc.vector.tensor_copy(out=rb, in_=rb_ps)
                        nc.vector.tensor_tensor(
                            at_sb[hb : hb + 64, mc, q0 : q0 + QB],
                            po[h][0:D, :],
                            rb,
                            mybir.AluOpType.mult,
                        )

        # ---- phase 3: output projection ------------------------------------
        with tc.tile_pool(name="ps3", bufs=3, space="PSUM") as ps3:
            for sc in range(SC):
                psy = ps3.tile([P, E], FP, tag="ps3")
                for nq in range(E // NQ):
                    for mc in range(MC):
                        nc.tensor.matmul(
                            psy[:, nq * NQ : (nq + 1) * NQ],
                            lhsT=at_sb[:, mc, sc * P : (sc + 1) * P],
                            rhs=wo_sb[:, mc, nq * NQ : (nq + 1) * NQ],
                            start=(mc == 0),
                            stop=(mc == MC - 1),
                        )
                y_sb = evac.tile([P, E], FP, tag="ysb")
                nc.vector.tensor_copy(out=y_sb, in_=psy)
                nc.sync.dma_start(out=y[sc * P : (sc + 1) * P, :], in_=y_sb)


_NC_CACHE = None


def build_nc() -> bass.Bass:
    global _NC_CACHE
    if _NC_CACHE is None:
        nc = bacc.Bacc(None, target_bir_lowering=False)
        with tile.TileContext(nc) as tc:
            _build_kernel(nc, tc)
        nc.compile()
        _NC_CACHE = nc
    return _NC_CACHE


def make_core_inputs(node: np.ndarray, W_qkv: np.ndarray, W_out: np.ndarray):
    """Shard full inputs into the 8 per-core input maps."""
    bf16 = ml_dtypes.bfloat16
    in_maps = []
    for c in range(NCORES):
        b, g = divmod(c, NCORES // B)
        sl = slice(g * GD, (g + 1) * GD)
        in_maps.append(
            {
                "xT": np.ascontiguousarray(node[b].T).astype(bf16),
                # fold the 1/sqrt(D) softmax scale into Wq (exact in bf16)
                "wq": np.ascontiguousarray(W_qkv[:, sl] * (1.0 / np.sqrt(D))).astype(
                    bf16
                ),
                "wk": np.ascontiguousarray(W_qkv[:, H * D + g * GD : H * D + (g + 1) * GD]).astype(bf16),
                "wv": np.ascontiguousarray(
                    W_qkv[:, 2 * H * D + g * GD : 2 * H * D + (g + 1) * GD]
                ).astype(bf16),
                "wo": np.ascontiguousarray(W_out[sl, :]).astype(bf16),
            }
        )
    return in_maps


def _run(node, W_qkv, W_out, **spmd_kwargs):
    nc = build_nc()
    in_maps = make_core_inputs(node, W_qkv, W_out)
    res = run_bass_kernel_spmd(
        nc, in_maps, core_ids=list(range(NCORES)), **spmd_kwargs
    )
    out = np.zeros((B, S, E), dtype=np.float32)
    for c in range(NCORES):
        b = c // (NCORES // B)
        out[b] += res.results[c]["y"]
    return out, res


def kernel(node: np.ndarray, W_qkv: np.ndarray, W_out: np.ndarray) -> np.ndarray:
    node = np.asarray(node, dtype=np.float32)
    W_qkv = np.asarray(W_qkv, dtype=np.float32)
    W_out = np.asarray(W_out, dtype=np.float32)
    out, _ = _run(node, W_qkv, W_out)
    return out


# revision 26
# speedup vs baseline: 1.1693x; 1.1693x over previous
"""Trainium2 Bass kernel for nn_AttentionKernel_89455578841177.

Multi-head attention: qkv = node @ W_qkv; softmax(q k^T / sqrt(D)) v; out @ W_out.
B=2, S=2048, E=1024, H=16, D=64.

Sharding over 8 NeuronCores: data parallel on B (2) x tensor parallel on heads
(16 heads -> 4 groups of 4). Each core computes a per-head-group partial of the
output projection; the host sums the 4 partials per batch element.

Device-side structure per core (all matmuls bf16 inputs, fp32 PSUM accumulate):
  phase 1: qT = (x Wq)^T, kT = (x Wk)^T  in [d, s] layout;  v = x Wv in [s, d]
           layout with a ones-column appended per head (for softmax row sums).
  phase 2: per head pair (row-group-tiled K=64 matmuls run concurrently):
           s^T = k q^T  -> exp on ScalarE -> p^T;   [o^T | r] = [v|1]^T-style
           matmul accumulated over k-chunks in PSUM;  normalize by 1/r.
  phase 3: y_partial = a W_out  (a = concatenated normalized heads).
The 1/sqrt(D) scale is folded into Wq on the host (exact: power of two).
Softmax skips the max-subtraction: scores are ~N(0,1) so exp cannot overflow.
"""

import numpy as np
import ml_dtypes

import concourse.bass as bass
import concourse.mybir as mybir
import concourse.tile as tile
from concourse import bacc
from concourse.bass_utils import run_bass_kernel_spmd

B, S, E = 2, 2048, 1024
H, D = 16, 64
NCORES = 8
GH = 4            # heads per core
GD = GH * D       # 256 = per-core slice of the head dim
P = 128
EO = E // P       # 8 contraction chunks for the projections
SC = S // P       # 16 s-chunks
MC = GD // P      # 2 head-pair chunks (2 heads of 64 rows per chunk)
NQ = 512          # matmul moving free dim
QB = 512          # Sq block size in phase 2
KV = D + 1        # v columns + ones column

BF = mybir.dt.bfloat16
FP = mybir.dt.float32
EXP = mybir.ActivationFunctionType.Exp

import os
_VARIANT = os.environ.get("KERNEL_VARIANT", "")


def _build_kernel(nc: bass.Bass, tc: tile.TileContext):
    xT = nc.dram_tensor("xT", [E, S], BF, kind="ExternalInput")
    wq = nc.dram_tensor("wq", [E, GD], BF, kind="ExternalInput")
    wk = nc.dram_tensor("wk", [E, GD], BF, kind="ExternalInput")
    wv = nc.dram_tensor("wv", [E, GD], BF, kind="ExternalInput")
    wo = nc.dram_tensor("wo", [GD, E], BF, kind="ExternalInput")
    sel = nc.dram_tensor("sel", [16, MC * (S // QB), P], BF, kind="ExternalInput")
    y = nc.dram_tensor("y", [S, E], FP, kind="ExternalOutput")

    with (
        tc.tile_pool(name="const", bufs=1) as const,
        tc.tile_pool(name="pwork", bufs=4) as pwork,
        tc.tile_pool(name="evac", bufs=3) as evac,
    ):
        # ---- SBUF residents -------------------------------------------------
        x_sb = const.tile([P, EO, S], BF, tag="x")
        xT_r = xT.rearrange("(eo p) s -> p eo s", p=P)
        for eo in range(EO):
            nc.sync.dma_start(out=x_sb[:, eo, :], in_=xT_r[:, eo, :])

        wq_sb = const.tile([P, EO, GD], BF, tag="wq")
        nc.sync.dma_start(out=wq_sb, in_=wq.rearrange("(eo p) m -> p eo m", p=P))
        wk_sb = const.tile([P, EO, GD], BF, tag="wk")
        nc.sync.dma_start(out=wk_sb, in_=wk.rearrange("(eo p) m -> p eo m", p=P))
        wv_sb = const.tile([P, EO, GD], BF, tag="wv")
        nc.sync.dma_start(out=wv_sb, in_=wv.rearrange("(eo p) m -> p eo m", p=P))
        wo_sb = const.tile([P, MC, E], BF, tag="wo")
        nc.sync.dma_start(out=wo_sb, in_=wo.rearrange("(mc p) e -> p mc e", p=P))

        qT_sb = const.tile([P, MC, S], BF, tag="qT")
        kT_sb = const.tile([P, MC, S], BF, tag="kT")
        at_sb = const.tile([P, MC, S], BF, tag="at")   # normalized attn out^T
        v_sb = const.tile([P, SC, GH, KV], BF, tag="v")
        nc.vector.memset(v_sb[:, :, :, D : D + 1], 1.0)
        # selector for broadcasting 1/r rows across partitions via matmul
        sel_sb = const.tile([16, MC * (S // QB), P], BF, tag="sel")
        nc.sync.dma_start(out=sel_sb, in_=sel[:])
        # softmax row sums for all (head, q-block): row = (mc*2+h)*4 + hf.
        # DVE writes must start at partition 0/32/64, so rows stage in
        # partition 0's free space and one DMA scatters them to 16 partitions.
        r1p = const.tile([1, 16, QB], FP, tag="r1p")
        rall = const.tile([16, QB], FP, tag="rall")

        # ---- phase 1: projections ------------------------------------------
        with tc.tile_pool(name="ps1", bufs=4, space="PSUM") as ps1:
            for wsrc, dst in ((wq_sb, qT_sb), (wk_sb, kT_sb)):
                for mc in range(MC):
                    for sq in range(S // NQ):
                        pst = ps1.tile([P, NQ], FP, tag="ps1")
                        for eo in range(EO):
                            nc.tensor.matmul(
                                pst,
                                lhsT=wsrc[:, eo, mc * P : (mc + 1) * P],
                                rhs=x_sb[:, eo, sq * NQ : (sq + 1) * NQ],
                                start=(eo == 0),
                                stop=(eo == EO - 1),
                            )
                        nc.vector.tensor_copy(
                            out=dst[:, mc, sq * NQ : (sq + 1) * NQ], in_=pst
                        )
            for sc in range(SC):
                psv = ps1.tile([P, NQ], FP, tag="ps1")
                for eo in range(EO):
                    nc.tensor.matmul(
                        psv[:, :GD],
                        lhsT=x_sb[:, eo, sc * P : (sc + 1) * P],
                        rhs=wv_sb[:, eo, :],
                        start=(eo == 0),
                        stop=(eo == EO - 1),
                    )
                nc.vector.tensor_copy(
                    out=v_sb[:, sc, :, 0:D],
                    in_=psv[:, :GD].rearrange("p (h d) -> p h d", h=GH),
                )

        # ---- phase 2: attention per head pair ------------------------------
        with (
            tc.tile_pool(name="ps_sc", bufs=2, space="PSUM") as ps_sc,
            tc.tile_pool(name="ps_pv", bufs=4, space="PSUM") as ps_pv,
        ):
            for mc in range(MC):
                for hf in range(S // QB):
                    q0 = hf * QB
                    po0 = ps_pv.tile([KV, QB], FP, tag="po")
                    po1 = ps_pv.tile([KV, QB], FP, tag="po")
                    po = (po0, po1)
                    for kc in range(SC):
                        # head pair packed side by side: one fp32 PSUM bank per
                        # head, K=64 row-tiled matmuls run concurrently
                        st = ps_sc.tile([P, 2 * QB], FP, tag="st")
                        for h in range(2):
                            hb = h * 64
                            nc.tensor.matmul(
                                st[:, h * QB : (h + 1) * QB],
                                lhsT=kT_sb[hb : hb + 64, mc, kc * P : (kc + 1) * P],
                                rhs=qT_sb[hb : hb + 64, mc, q0 : q0 + QB],
                                start=True,
                                stop=True,
                            )
                        pt = pwork.tile([P, 2 * QB], BF, tag="p")
                        nc.scalar.activation(pt, st, EXP)
                        for h in range(2):
                            nc.tensor.matmul(
                                po[h],
                                lhsT=v_sb[:, kc, mc * 2 + h, :],
                                rhs=pt[:, h * QB : (h + 1) * QB],
                                start=(kc == 0),
                                stop=(kc == SC - 1),
                                skip_group_check=True,
                            )
                    # evacuate PSUM immediately (unnormalized) so the po slots
                    # recycle without waiting on the normalization chain
                    for h in range(2):
                        hb = h * 64
                        nc.vector.tensor_copy(
                            out=at_sb[hb : hb + 64, mc, q0 : q0 + QB],
                            in_=po[h][0:D, :],
                        )
                        ridx = (mc * 2 + h) * (S // QB) + hf
                        nc.vector.tensor_copy(
                            out=r1p[0:1, ridx, :], in_=po[h][D : D + 1, :]
                        )

        # ---- normalization: one batched reciprocal, lazy per-block scale ---
        # partition-0 staging -> DRAM bounce -> 16 partitions (a direct
        # SBUF->SBUF partition scatter garbles data on HW)
        with tc.tile_pool(name="dscr", bufs=1, space="DRAM") as dscr:
            rbounce = dscr.tile([16, QB], FP, tag="rbounce")
            nc.sync.dma_start(
                out=rbounce[:].rearrange("a b -> () a b"), in_=r1p
            )
            nc.sync.dma_start(out=rall, in_=rbounce[:])
        rinv_all = const.tile([16, QB], BF, tag="rinvall")
        with nc.allow_low_precision(reason="1/rowsum scalar; bf16 is plenty"):
            nc.vector.reciprocal(rinv_all, rall)

        # ---- phase 3: output projection ------------------------------------
        with (
            tc.tile_pool(name="ps_rb", bufs=2, space="PSUM") as ps_rb,
            tc.tile_pool(name="ps3", bufs=3, space="PSUM") as ps3,
        ):
            # apply 1/r to the staged attention outputs (in place); the
            # broadcast of both heads' 1/r rows across 128 partitions is a
            # single K=16 selector matmul per (mc, q-block)
            for mc in range(MC):
                for hf in range(S // QB):
                    q0 = hf * QB
                    j = mc * (S // QB) + hf
                    rb_ps = ps_rb.tile([P, QB], FP, tag="rb")
                    nc.tensor.matmul(
                        rb_ps,
                        lhsT=sel_sb[:, j, :],
                        rhs=rinv_all,
                        start=True,
                        stop=True,
                    )
                    nc.vector.tensor_tensor(
                        at_sb[:, mc, q0 : q0 + QB],
                        at_sb[:, mc, q0 : q0 + QB],
                        rb_ps,
                        mybir.AluOpType.mult,
                    )
            for sc in range(SC):
                psy = ps3.tile([P, E], FP, tag="ps3")
                for nq in range(E // NQ):
                    for mc in range(MC):
                        nc.tensor.matmul(
                            psy[:, nq * NQ : (nq + 1) * NQ],
                            lhsT=at_sb[:, mc, sc * P : (sc + 1) * P],
                            rhs=wo_sb[:, mc, nq * NQ : (nq + 1) * NQ],
                            start=(mc == 0),
                            stop=(mc == MC - 1),
                        )
                y_sb = evac.tile([P, E], FP, tag="ysb")
                nc.vector.tensor_copy(out=y_sb, in_=psy)
                nc.sync.dma_start(out=y[sc * P : (sc + 1) * P, :], in_=y_sb)


_NC_CACHE = None


def build_nc() -> bass.Bass:
    global _NC_CACHE
    if _NC_CACHE is None:
        nc = bacc.Bacc(None, target_bir_lowering=False)
        with tile.TileContext(nc) as tc:
            _build_kernel(nc, tc)
        nc.compile()
        _NC_CACHE = nc
    return _NC_CACHE


def _make_sel() -> np.ndarray:
    """Selector weights: sel[k, j, p] = 1 where k == 1/r row for (j, p)."""
    nhf = S // QB
    sel = np.zeros((16, MC * nhf, P), dtype=np.float32)
    for mc in range(MC):
        for hf in range(nhf):
            j = mc * nhf + hf
            for h in range(2):
                row = (mc * 2 + h) * nhf + hf
                sel[row, j, h * 64 : (h + 1) * 64] = 1.0
    return sel.astype(ml_dtypes.bfloat16)


def make_core_inputs(node: np.ndarray, W_qkv: np.ndarray, W_out: np.ndarray):
    """Shard full inputs into the 8 per-core input maps."""
    bf16 = ml_dtypes.bfloat16
    sel = _make_sel()
    in_maps = []
    for c in range(NCORES):
        b, g = divmod(c, NCORES // B)
        sl = slice(g * GD, (g + 1) * GD)
        in_maps.append(
            {
                "xT": np.ascontiguousarray(node[b].T).astype(bf16),
                # fold the 1/sqrt(D) softmax scale into Wq (exact in bf16)
                "wq": np.ascontiguousarray(W_qkv[:, sl] * (1.0 / np.sqrt(D))).astype(
                    bf16
                ),
                "wk": np.ascontiguousarray(W_qkv[:, H * D + g * GD : H * D + (g + 1) * GD]).astype(bf16),
                "wv": np.ascontiguousarray(
                    W_qkv[:, 2 * H * D + g * GD : 2 * H * D + (g + 1) * GD]
                ).astype(bf16),
                "wo": np.ascontiguousarray(W_out[sl, :]).astype(bf16),
                "sel": sel,
            }
        )
    return in_maps


def _run(node, W_qkv, W_out, **spmd_kwargs):
    nc = build_nc()
    in_maps = make_core_inputs(node, W_qkv, W_out)
    res = run_bass_kernel_spmd(
        nc, in_maps, core_ids=list(range(NCORES)), **spmd_kwargs
    )
    out = np.zeros((B, S, E), dtype=np.float32)
    for c in range(NCORES):
        b = c // (NCORES // B)
        out[b] += res.results[c]["y"]
    return out, res


def kernel(node: np.ndarray, W_qkv: np.ndarray, W_out: np.ndarray) -> np.ndarray:
    node = np.asarray(node, dtype=np.float32)
    W_qkv = np.asarray(W_qkv, dtype=np.float32)
    W_out = np.asarray(W_out, dtype=np.float32)
    out, _ = _run(node, W_qkv, W_out)
    return out


# revision 27
# speedup vs baseline: 1.2025x; 1.0284x over previous
"""Trainium2 Bass kernel for nn_AttentionKernel_89455578841177.

Multi-head attention: qkv = node @ W_qkv; softmax(q k^T / sqrt(D)) v; out @ W_out.
B=2, S=2048, E=1024, H=16, D=64.

Sharding over 8 NeuronCores: data parallel on B (2) x tensor parallel on heads
(16 heads -> 4 groups of 4). Each core computes a per-head-group partial of the
output projection; the host sums the 4 partials per batch element.

Device-side structure per core (all matmuls bf16 inputs, fp32 PSUM accumulate):
  phase 1: qT = (x Wq)^T, kT = (x Wk)^T in [d, s] layout (weights stationary).
  main loop over q-blocks (hf) x head pairs (mc), ScalarE-exp-bound:
    s^T = k q^T (two K=64 row-tiled matmuls run concurrently) -> exp -> p^T
    [o^T | r] accumulated over k-chunks in PSUM via [v | ones] stationary.
    v itself is projected inside the first q-block's k-loop (x stationary).
    After each (hf): batched approx-reciprocal of the 4 row-sum vectors,
    broadcast across partitions via a DRAM bounce, in-place scale of o^T,
    then that q-block's slice of the output projection y = a W_out.
The 1/sqrt(D) scale is folded into Wq on the host (exact: power of two).
Softmax skips the max-subtraction: scores are ~N(0,1) so exp cannot overflow.
"""

import os

import numpy as np
import ml_dtypes

import concourse.bass as bass
import concourse.mybir as mybir
import concourse.tile as tile
from concourse import bacc
from concourse.bass_utils import run_bass_kernel_spmd

B, S, E = 2, 2048, 1024
H, D = 16, 64
NCORES = 8
GH = 4            # heads per core
GD = GH * D       # 256 = per-core slice of the head dim
P = 128
EO = E // P       # 8 contraction chunks for the projections
SC = S // P       # 16 s-chunks
MC = GD // P      # 2 head-pair chunks (2 heads of 64 rows per chunk)
NQ = 512          # matmul moving free dim
QB = 512          # Sq block size in the attention loop
NHF = S // QB     # 4 q-blocks
KV = D + 1        # v columns + ones column

BF = mybir.dt.bfloat16
FP = mybir.dt.float32
EXP = mybir.ActivationFunctionType.Exp


def _build_kernel(nc: bass.Bass, tc: tile.TileContext):
    xT = nc.dram_tensor("xT", [E, S], BF, kind="ExternalInput")
    wq = nc.dram_tensor("wq", [E, GD], BF, kind="ExternalInput")
    wk = nc.dram_tensor("wk", [E, GD], BF, kind="ExternalInput")
    wv = nc.dram_tensor("wv", [E, GD], BF, kind="ExternalInput")
    wo = nc.dram_tensor("wo", [GD, E], BF, kind="ExternalInput")
    y = nc.dram_tensor("y", [S, E], FP, kind="ExternalOutput")

    with (
        tc.tile_pool(name="const", bufs=1) as const,
        tc.tile_pool(name="pwork", bufs=4) as pwork,
        tc.tile_pool(name="evac", bufs=3) as evac,
        tc.tile_pool(name="dscr", bufs=2, space="DRAM") as dscr,
    ):
        # ---- SBUF residents -------------------------------------------------
        x_sb = const.tile([P, EO, S], BF, tag="x")
        xT_r = xT.rearrange("(eo p) s -> p eo s", p=P)
        for eo in range(EO):
            nc.sync.dma_start(out=x_sb[:, eo, :], in_=xT_r[:, eo, :])

        wq_sb = const.tile([P, EO, GD], BF, tag="wq")
        nc.sync.dma_start(out=wq_sb, in_=wq.rearrange("(eo p) m -> p eo m", p=P))
        wk_sb = const.tile([P, EO, GD], BF, tag="wk")
        nc.sync.dma_start(out=wk_sb, in_=wk.rearrange("(eo p) m -> p eo m", p=P))
        wv_sb = const.tile([P, EO, GD], BF, tag="wv")
        nc.sync.dma_start(out=wv_sb, in_=wv.rearrange("(eo p) m -> p eo m", p=P))
        wo_sb = const.tile([P, MC, E], BF, tag="wo")
        nc.sync.dma_start(out=wo_sb, in_=wo.rearrange("(mc p) e -> p mc e", p=P))

        qT_sb = const.tile([P, MC, S], BF, tag="qT")
        kT_sb = const.tile([P, MC, S], BF, tag="kT")
        at_sb = const.tile([P, MC, S], BF, tag="at")   # attn out^T (unnorm->norm)
        v_sb = const.tile([P, SC, GH, KV], BF, tag="v")
        nc.vector.memset(v_sb[:, :, :, D : D + 1], 1.0)
        # softmax row-sum staging: DVE writes must start at partition 0/32/64,
        # so rows live in partition 0's free space; row = hf*4 + mc*2 + h
        r1p = const.tile([1, 4 * NHF, QB], FP, tag="r1p")

        # ---- phase 1: qT / kT projections (weights stationary) -------------
        with tc.tile_pool(name="ps1", bufs=4, space="PSUM") as ps1:
            for wsrc, dst in ((wq_sb, qT_sb), (wk_sb, kT_sb)):
                for mc in range(MC):
                    psts = [
                        ps1.tile([P, NQ], FP, tag="ps1", name=f"pst{sq}")
                        for sq in range(S // NQ)
                    ]
                    for eo in range(EO):
                        for sq in range(S // NQ):
                            nc.tensor.matmul(
                                psts[sq],
                                lhsT=wsrc[:, eo, mc * P : (mc + 1) * P],
                                rhs=x_sb[:, eo, sq * NQ : (sq + 1) * NQ],
                                start=(eo == 0),
                                stop=(eo == EO - 1),
                            )
                    for sq in range(S // NQ):
                        nc.vector.tensor_copy(
                            out=dst[:, mc, sq * NQ : (sq + 1) * NQ], in_=psts[sq]
                        )

        # ---- main loop: attention + inline v-proj + inline out-proj --------
        with (
            tc.tile_pool(name="ps_sc", bufs=2, space="PSUM") as ps_sc,
            tc.tile_pool(name="ps_pv", bufs=2, space="PSUM") as ps_pv,
            tc.tile_pool(name="ps3", bufs=2, space="PSUM") as ps3,
        ):
            for hf in range(NHF):
                q0 = hf * QB
                for mc in range(MC):
                    po0 = ps_pv.tile([KV, QB], FP, tag="po")
                    po1 = ps_pv.tile([KV, QB], FP, tag="po")
                    po = (po0, po1)
                    for kc in range(SC):
                        if hf == 0 and mc == 0:
                            # v projection for this k-chunk (all 4 heads)
                            psv = ps3.tile([P, NQ], FP, tag="ps3", name="psv")
                            for eo in range(EO):
                                nc.tensor.matmul(
                                    psv[:, :GD],
                                    lhsT=x_sb[:, eo, kc * P : (kc + 1) * P],
                                    rhs=wv_sb[:, eo, :],
                                    start=(eo == 0),
                                    stop=(eo == EO - 1),
                                )
                            nc.vector.tensor_copy(
                                out=v_sb[:, kc, :, 0:D],
                                in_=psv[:, :GD].rearrange("p (h d) -> p h d", h=GH),
                            )
                        # head pair packed side by side, one fp32 bank per
                        # head; K=64 row-tiled matmuls run concurrently
                        st = ps_sc.tile([P, 2 * QB], FP, tag="st")
                        for h in range(2):
                            hb = h * 64
                            nc.tensor.matmul(
                                st[:, h * QB : (h + 1) * QB],
                                lhsT=kT_sb[hb : hb + 64, mc, kc * P : (kc + 1) * P],
                                rhs=qT_sb[hb : hb + 64, mc, q0 : q0 + QB],
                                start=True,
                                stop=True,
                            )
                        pt = pwork.tile([P, 2 * QB], BF, tag="p")
                        nc.scalar.activation(pt, st, EXP)
                        for h in range(2):
                            nc.tensor.matmul(
                                po[h],
                                lhsT=v_sb[:, kc, mc * 2 + h, :],
                                rhs=pt[:, h * QB : (h + 1) * QB],
                                start=(kc == 0),
                                stop=(kc == SC - 1),
                                skip_group_check=True,
                            )
                    # evacuate PSUM immediately so the po slots recycle
                    for h in range(2):
                        hb = h * 64
                        nc.vector.tensor_copy(
                            out=at_sb[hb : hb + 64, mc, q0 : q0 + QB],
                            in_=po[h][0:D, :],
                        )
                        ridx = hf * 4 + mc * 2 + h
                        nc.vector.tensor_copy(
                            out=r1p[0:1, ridx, :], in_=po[h][D : D + 1, :]
                        )

                # normalization for this q-block: rows -> DRAM -> partitions,
                # batched approx reciprocal, broadcast back via DRAM source
                # APs with zero partition step (groupnorm-style)
                rd4 = dscr.tile([4, QB], FP, tag="rd4")
                nc.sync.dma_start(
                    out=rd4[:].rearrange("a b -> () a b"),
                    in_=r1p[0:1, hf * 4 : (hf + 1) * 4, :],
                )
                rall4 = evac.tile([4, QB], FP, tag="rall4")
                nc.sync.dma_start(out=rall4, in_=rd4[:])
                rinv4 = evac.tile([4, QB], FP, tag="rinv4")
                nc.vector.reciprocal_approx_fast(rinv4, rall4)
                rid4 = dscr.tile([4, QB], FP, tag="rid4")
                nc.sync.dma_start(out=rid4[:], in_=rinv4)
                for mc in range(MC):
                    rb = evac.tile([P, QB], BF, tag="rb")
                    for h in range(2):
                        row = rid4[mc * 2 + h : mc * 2 + h + 1, :]
                        bc = bass.AP(
                            tensor=row.tensor,
                            offset=row.offset,
                            ap=[[0, 64]] + [list(dim) for dim in row.ap[1:]],
                        )
                        nc.gpsimd.dma_start(out=rb[h * 64 : (h + 1) * 64, :], in_=bc)
                    nc.vector.tensor_tensor(
                        at_sb[:, mc, q0 : q0 + QB],
                        at_sb[:, mc, q0 : q0 + QB],
                        rb,
                        mybir.AluOpType.mult,
                    )

                # output projection for this q-block
                for sc in range(hf * (QB // P), (hf + 1) * (QB // P)):
                    y_sb = evac.tile([P, E], FP, tag="ysb")
                    for nq in range(E // NQ):
                        psy = ps3.tile([P, NQ], FP, tag="ps3", name="psy")
                        for mc in range(MC):
                            nc.tensor.matmul(
                                psy,
                                lhsT=at_sb[:, mc, sc * P : (sc + 1) * P],
                                rhs=wo_sb[:, mc, nq * NQ : (nq + 1) * NQ],
                                start=(mc == 0),
                                stop=(mc == MC - 1),
                            )
                        nc.vector.tensor_copy(
                            out=y_sb[:, nq * NQ : (nq + 1) * NQ], in_=psy
                        )
                    nc.sync.dma_start(out=y[sc * P : (sc + 1) * P, :], in_=y_sb)


_NC_CACHE = None


def build_nc() -> bass.Bass:
    global _NC_CACHE
    if _NC_CACHE is None:
        nc = bacc.Bacc(None, target_bir_lowering=False)
        with tile.TileContext(nc) as tc:
            _build_kernel(nc, tc)
        nc.compile()
        _NC_CACHE = nc
    return _NC_CACHE


def make_core_inputs(node: np.ndarray, W_qkv: np.ndarray, W_out: np.ndarray):
    """Shard full inputs into the 8 per-core input maps."""
    bf16 = ml_dtypes.bfloat16
    in_maps = []
    for c in range(NCORES):
        b, g = divmod(c, NCORES // B)
        sl = slice(g * GD, (g + 1) * GD)
        in_maps.append(
            {
                "xT": np.ascontiguousarray(node[b].T).astype(bf16),
                # fold the 1/sqrt(D) softmax scale into Wq (exact in bf16)
                "wq": np.ascontiguousarray(W_qkv[:, sl] * (1.0 / np.sqrt(D))).astype(
                    bf16
                ),
                "wk": np.ascontiguousarray(
                    W_qkv[:, H * D + g * GD : H * D + (g + 1) * GD]
                ).astype(bf16),
                "wv": np.ascontiguousarray(
                    W_qkv[:, 2 * H * D + g * GD : 2 * H * D + (g + 1) * GD]
                ).astype(bf16),
                "wo": np.ascontiguousarray(W_out[sl, :]).astype(bf16),
            }
        )
    return in_maps


def _run(node, W_qkv, W_out, **spmd_kwargs):
    nc = build_nc()
    in_maps = make_core_inputs(node, W_qkv, W_out)
    res = run_bass_kernel_spmd(
        nc, in_maps, core_ids=list(range(NCORES)), **spmd_kwargs
    )
    out = np.zeros((B, S, E), dtype=np.float32)
    for c in range(NCORES):
        b = c // (NCORES // B)
        out[b] += res.results[c]["y"]
    return out, res


def kernel(node: np.ndarray, W_qkv: np.ndarray, W_out: np.ndarray) -> np.ndarray:
    node = np.asarray(node, dtype=np.float32)
    W_qkv = np.asarray(W_qkv, dtype=np.float32)
    W_out = np.asarray(W_out, dtype=np.float32)
    out, _ = _run(node, W_qkv, W_out)
    return out


# revision 29
# speedup vs baseline: 1.2296x; 1.0226x over previous
"""Trainium2 Bass kernel for nn_AttentionKernel_89455578841177.

Multi-head attention: qkv = node @ W_qkv; softmax(q k^T / sqrt(D)) v; out @ W_out.
B=2, S=2048, E=1024, H=16, D=64.

Sharding over 8 NeuronCores: data parallel on B (2) x tensor parallel on heads
(16 heads -> 4 groups of 4). Each core computes a per-head-group partial of the
output projection; the host sums the 4 partials per batch element.

Device-side structure per core (all matmuls bf16 inputs, fp32 PSUM accumulate):
  phase 1: qT = (x Wq)^T, kT = (x Wk)^T in [d, s] layout (weights stationary).
  main loop over q-blocks (hf) x head pairs (mc), ScalarE-exp-bound:
    s^T = k q^T (two K=64 row-tiled matmuls run concurrently) -> exp -> p^T
    [o^T | r] accumulated over k-chunks in PSUM via [v | ones] stationary.
    v itself is projected inside the first q-block's k-loop (x stationary).
    After each (hf): batched approx-reciprocal of the 4 row-sum vectors,
    broadcast across partitions via a DRAM bounce, in-place scale of o^T,
    then that q-block's slice of the output projection y = a W_out.
The 1/sqrt(D) scale is folded into Wq on the host (exact: power of two).
Softmax skips the max-subtraction: scores are ~N(0,1) so exp cannot overflow.
"""

import os

import numpy as np
import ml_dtypes

import concourse.bass as bass
import concourse.mybir as mybir
import concourse.tile as tile
from concourse import bacc
from concourse.bass_utils import run_bass_kernel_spmd

B, S, E = 2, 2048, 1024
H, D = 16, 64
NCORES = 8
GH = 4            # heads per core
GD = GH * D       # 256 = per-core slice of the head dim
P = 128
EO = E // P       # 8 contraction chunks for the projections
SC = S // P       # 16 s-chunks
MC = GD // P      # 2 head-pair chunks (2 heads of 64 rows per chunk)
NQ = 512          # matmul moving free dim
QB = 512          # Sq block size in the attention loop
NHF = S // QB     # 4 q-blocks
KV = D + 1        # v columns + ones column

BF = mybir.dt.bfloat16
FP = mybir.dt.float32
EXP = mybir.ActivationFunctionType.Exp


def _build_kernel(nc: bass.Bass, tc: tile.TileContext):
    xT = nc.dram_tensor("xT", [E, S], BF, kind="ExternalInput")
    wq = nc.dram_tensor("wq", [E, GD], BF, kind="ExternalInput")
    wk = nc.dram_tensor("wk", [E, GD], BF, kind="ExternalInput")
    wv = nc.dram_tensor("wv", [E, GD], BF, kind="ExternalInput")
    wo = nc.dram_tensor("wo", [GD, E], BF, kind="ExternalInput")
    y = nc.dram_tensor("y", [S, E], FP, kind="ExternalOutput")

    with (
        tc.tile_pool(name="const", bufs=1) as const,
        tc.tile_pool(name="pwork", bufs=4) as pwork,
        tc.tile_pool(name="evac", bufs=3) as evac,
    ):
        # ---- SBUF residents -------------------------------------------------
        x_sb = const.tile([P, EO, S], BF, tag="x")
        xT_r = xT.rearrange("(eo p) s -> p eo s", p=P)
        for eo in range(EO):
            nc.sync.dma_start(out=x_sb[:, eo, :], in_=xT_r[:, eo, :])

        wq_sb = const.tile([P, EO, GD], BF, tag="wq")
        nc.sync.dma_start(out=wq_sb, in_=wq.rearrange("(eo p) m -> p eo m", p=P))
        wk_sb = const.tile([P, EO, GD], BF, tag="wk")
        nc.sync.dma_start(out=wk_sb, in_=wk.rearrange("(eo p) m -> p eo m", p=P))
        wv_sb = const.tile([P, EO, GD], BF, tag="wv")
        nc.sync.dma_start(out=wv_sb, in_=wv.rearrange("(eo p) m -> p eo m", p=P))
        wo_sb = const.tile([P, MC, E], BF, tag="wo")
        nc.sync.dma_start(out=wo_sb, in_=wo.rearrange("(mc p) e -> p mc e", p=P))

        qT_sb = const.tile([P, MC, S], BF, tag="qT")
        kT_sb = const.tile([P, MC, S], BF, tag="kT")
        at_sb = const.tile([P, MC, S], BF, tag="at")   # attn out^T (unnorm->norm)
        v_sb = const.tile([P, SC, GH, KV], BF, tag="v")
        nc.vector.memset(v_sb[:, :, :, D : D + 1], 1.0)
        # ones column for broadcasting 1/r rows across partitions via K=1 mm
        ones_b = const.tile([1, 64], BF, tag="ones")
        nc.vector.memset(ones_b, 1.0)

        # one PSUM bank budget for everything: scores pair (2 banks x2 bufs),
        # [o^T|r] accumulators (1 bank x2), and a shared 1-bank pool for the
        # projections / broadcasts (x2) = 8 banks exactly
        with (
            tc.tile_pool(name="ps_sc", bufs=2, space="PSUM") as ps_sc,
            tc.tile_pool(name="ps_pv", bufs=2, space="PSUM") as ps_pv,
            tc.tile_pool(name="psq", bufs=2, space="PSUM") as psq,
        ):
            # ---- phase 1: qT / kT projections (weights stationary) ---------
            # mc-major emission so attention on the first head pair can start
            # while the second pair's projections still run on the PE
            for mc in range(MC):
                for wsrc, dst in ((wq_sb, qT_sb), (wk_sb, kT_sb)):
                    for sqb in range(2):
                        psts = [
                            psq.tile([P, NQ], FP, tag="sq", name=f"pst{sq}")
                            for sq in range(2)
                        ]
                        for eo in range(EO):
                            for sq in range(2):
                                s0 = (sqb * 2 + sq) * NQ
                                nc.tensor.matmul(
                                    psts[sq],
                                    lhsT=wsrc[:, eo, mc * P : (mc + 1) * P],
                                    rhs=x_sb[:, eo, s0 : s0 + NQ],
                                    start=(eo == 0),
                                    stop=(eo == EO - 1),
                                )
                        for sq in range(2):
                            s0 = (sqb * 2 + sq) * NQ
                            nc.vector.tensor_copy(
                                out=dst[:, mc, s0 : s0 + NQ], in_=psts[sq]
                            )

            # ---- main loop: attention + inline v-proj + inline out-proj ----
            for hf in range(NHF):
                q0 = hf * QB
                # per-(mc*2+h) softmax row sums, staged in partition 0
                r4 = evac.tile([1, 4, QB], FP, tag="r4", bufs=2)
                for mc in range(MC):
                    po0 = ps_pv.tile([KV, QB], FP, tag="po")
                    po1 = ps_pv.tile([KV, QB], FP, tag="po")
                    po = (po0, po1)
                    for kc in range(SC):
                        if hf == 0 and mc == 0:
                            # v projection for this k-chunk (all 4 heads)
                            psv = psq.tile([P, NQ], FP, tag="sq", name="psv")
                            for eo in range(EO):
                                nc.tensor.matmul(
                                    psv[:, :GD],
                                    lhsT=x_sb[:, eo, kc * P : (kc + 1) * P],
                                    rhs=wv_sb[:, eo, :],
                                    start=(eo == 0),
                                    stop=(eo == EO - 1),
                                )
                            nc.vector.tensor_copy(
                                out=v_sb[:, kc, :, 0:D],
                                in_=psv[:, :GD].rearrange("p (h d) -> p h d", h=GH),
                            )
                        # head pair packed side by side, one fp32 bank per
                        # head; K=64 row-tiled matmuls run concurrently
                        st = ps_sc.tile([P, 2 * QB], FP, tag="st")
                        for h in range(2):
                            hb = h * 64
                            nc.tensor.matmul(
                                st[:, h * QB : (h + 1) * QB],
                                lhsT=kT_sb[hb : hb + 64, mc, kc * P : (kc + 1) * P],
                                rhs=qT_sb[hb : hb + 64, mc, q0 : q0 + QB],
                                start=True,
                                stop=True,
                            )
                        pt = pwork.tile([P, 2 * QB], BF, tag="p")
                        nc.scalar.activation(pt, st, EXP)
                        for h in range(2):
                            nc.tensor.matmul(
                                po[h],
                                lhsT=v_sb[:, kc, mc * 2 + h, :],
                                rhs=pt[:, h * QB : (h + 1) * QB],
                                start=(kc == 0),
                                stop=(kc == SC - 1),
                                skip_group_check=True,
                            )
                    # evacuate PSUM immediately so the po slots recycle
                    for h in range(2):
                        hb = h * 64
                        nc.vector.tensor_copy(
                            out=at_sb[hb : hb + 64, mc, q0 : q0 + QB],
                            in_=po[h][0:D, :],
                        )
                        nc.vector.tensor_copy(
                            out=r4[0:1, mc * 2 + h, :], in_=po[h][D : D + 1, :]
                        )

                # normalization, all on-chip: batched approx reciprocal on the
                # partition-0 staging rows, cast to bf16, then broadcast each
                # row across 64 partitions with a K=1 ones matmul
                rinv4 = evac.tile([1, 4, QB], FP, tag="rinv4", bufs=2)
                nc.vector.reciprocal_approx_fast(rinv4, r4)
                rinvb = evac.tile([1, 4, QB], BF, tag="rinvb", bufs=2)
                nc.vector.tensor_copy(out=rinvb, in_=rinv4)
                for mc in range(MC):
                    rb_ps = psq.tile([P, QB], FP, tag="sq", name="rb")
                    for h in range(2):
                        nc.tensor.matmul(
                            rb_ps[h * 64 : (h + 1) * 64, :],
                            lhsT=ones_b,
                            rhs=rinvb[0:1, mc * 2 + h, :],
                            start=True,
                            stop=True,
                        )
                    nc.vector.tensor_tensor(
                        at_sb[:, mc, q0 : q0 + QB],
                        at_sb[:, mc, q0 : q0 + QB],
                        rb_ps,
                        mybir.AluOpType.mult,
                    )

                # output projection for this q-block
                for sc in range(hf * (QB // P), (hf + 1) * (QB // P)):
                    y_sb = evac.tile([P, E], FP, tag="ysb")
                    for nq in range(E // NQ):
                        psy = psq.tile([P, NQ], FP, tag="sq", name="psy")
                        for mc in range(MC):
                            nc.tensor.matmul(
                                psy,
                                lhsT=at_sb[:, mc, sc * P : (sc + 1) * P],
                                rhs=wo_sb[:, mc, nq * NQ : (nq + 1) * NQ],
                                start=(mc == 0),
                                stop=(mc == MC - 1),
                            )
                        nc.vector.tensor_copy(
                            out=y_sb[:, nq * NQ : (nq + 1) * NQ], in_=psy
                        )
                    nc.sync.dma_start(out=y[sc * P : (sc + 1) * P, :], in_=y_sb)


_NC_CACHE = None


def build_nc() -> bass.Bass:
    global _NC_CACHE
    if _NC_CACHE is None:
        nc = bacc.Bacc(None, target_bir_lowering=False)
        with tile.TileContext(nc) as tc:
            _build_kernel(nc, tc)
        nc.compile()
        _NC_CACHE = nc
    return _NC_CACHE


def make_core_inputs(node: np.ndarray, W_qkv: np.ndarray, W_out: np.ndarray):
    """Shard full inputs into the 8 per-core input maps."""
    bf16 = ml_dtypes.bfloat16
    in_maps = []
    for c in range(NCORES):
        b, g = divmod(c, NCORES // B)
        sl = slice(g * GD, (g + 1) * GD)
        in_maps.append(
            {
                "xT": np.ascontiguousarray(node[b].T).astype(bf16),
                # fold the 1/sqrt(D) softmax scale into Wq (exact in bf16)
                "wq": np.ascontiguousarray(W_qkv[:, sl] * (1.0 / np.sqrt(D))).astype(
                    bf16
                ),
                "wk": np.ascontiguousarray(
                    W_qkv[:, H * D + g * GD : H * D + (g + 1) * GD]
                ).astype(bf16),
                "wv": np.ascontiguousarray(
                    W_qkv[:, 2 * H * D + g * GD : 2 * H * D + (g + 1) * GD]
                ).astype(bf16),
                "wo": np.ascontiguousarray(W_out[sl, :]).astype(bf16),
            }
        )
    return in_maps


def _run(node, W_qkv, W_out, **spmd_kwargs):
    nc = build_nc()
    in_maps = make_core_inputs(node, W_qkv, W_out)
    res = run_bass_kernel_spmd(
        nc, in_maps, core_ids=list(range(NCORES)), **spmd_kwargs
    )
    out = np.zeros((B, S, E), dtype=np.float32)
    for c in range(NCORES):
        b = c // (NCORES // B)
        out[b] += res.results[c]["y"]
    return out, res


def kernel(node: np.ndarray, W_qkv: np.ndarray, W_out: np.ndarray) -> np.ndarray:
    node = np.asarray(node, dtype=np.float32)
    W_qkv = np.asarray(W_qkv, dtype=np.float32)
    W_out = np.asarray(W_out, dtype=np.float32)
    out, _ = _run(node, W_qkv, W_out)
    return out
